# revision 1
# baseline (speedup 1.0000x reference)
"""MHC residual mixer: out[b,i,t,d] = sum_j H[i,j] * streams[b,j,t,d],
H = sinkhorn(logits). Streams mixed on-device; Sinkhorn (8x8, 20 iters) on host.

Sharding: 8 cores, core c handles batch b=c//2, T-half c%2 -> per-core
x[8, 1024, 1024] f32 (32 MiB). The stream-mix becomes a [128,128] stationary
matmul by packing (stream j, group g) on partitions and using a block-diagonal
weight W[j*16+g, i*16+g] = H[i,j].
"""

import os
import sys
import types
import numpy as np

import concourse.bass as bass
import concourse.mybir as mybir
from concourse import bacc
from concourse import bass_utils
from concourse.tile import TileContext


def _install_ntff_hook():
    # The image's `antenv` package lacks `axon_hooks`, so bass_utils'
    # trace path can't find the NTFF profile hook. Recreate it from the
    # boot shim's ctypes factory. Only needed when profiling (MIX_TRACE=1).
    if "antenv.axon_hooks" in sys.modules:
        return
    try:
        import antenv
        from trn_agent_boot.trn_boot import _ntff_profile_via_ctypes

        hook = _ntff_profile_via_ctypes("/opt/axon/libaxon_pjrt.so")
        mod = types.ModuleType("antenv.axon_hooks")
        mod.get_axon_ntff_profile_hook = lambda: hook
        mod.set_axon_ntff_profile_hook = lambda h: None
        sys.modules["antenv.axon_hooks"] = mod
        antenv.axon_hooks = mod
    except Exception as e:  # profiling is best-effort; execution still works
        print(f"ntff hook install failed: {e}", file=sys.stderr)

B, N, T, D = 4, 8, 2048, 1024
TH = T // 2                      # per-core T slice
POS = TH * D                     # positions per core per stream = 1,048,576
G = 16                           # groups on partitions (N*G = 128)
F = 4096                         # free columns per SBUF tile
MM_N = 512                       # fp32 matmul max moving free dim
NT = POS // (G * F)              # tiles per core
SINKHORN_ITERS = 20
TEMPERATURE = 1.0
EPS = np.float32(1e-8)
F32 = mybir.dt.float32
BF16 = mybir.dt.float16
USE_BF16 = os.environ.get("MIX_BF16", "0") == "1"

_cache = {}


def _sinkhorn_np(logits):
    x = logits.astype(np.float32)
    x = x - x.max(axis=-1, keepdims=True)
    p = np.exp(x) + EPS
    for _ in range(SINKHORN_ITERS):
        p = p / (p.sum(axis=-1, keepdims=True) + EPS)
        p = p / (p.sum(axis=-2, keepdims=True) + EPS)
    return p.astype(np.float32)


def _expand_w(H):
    # W[j*G+g, i*G+g] = H[i, j]  so that  out = W.T @ x  mixes streams per group
    Wm = np.zeros((128, 128), dtype=np.float32)
    g = np.arange(G)
    for j in range(N):
        for i in range(N):
            Wm[j * G + g, i * G + g] = H[i, j]
    return Wm


def _build_nc():
    nc = bacc.Bacc(
        "TRN2", target_bir_lowering=False, debug=False, enable_asserts=False
    )
    x = nc.dram_tensor("x", [N, TH, D], F32, kind="ExternalInput").ap()
    if USE_BF16:
        wh = nc.dram_tensor("wh", [128, 128], BF16, kind="ExternalInput").ap()
        wl = nc.dram_tensor("wl", [128, 128], BF16, kind="ExternalInput").ap()
    else:
        w = nc.dram_tensor("w", [128, 128], F32, kind="ExternalInput").ap()
    y = nc.dram_tensor("y", [N, TH, D], F32, kind="ExternalOutput").ap()

    # g-major position layout: position = g*(NT*F) + c*F + f. The 16 g-chunks
    # per stream are non-adjacent in DRAM, so each per-tile DMA lowers to
    # 128 descriptors of F*4 bytes (8 KB) instead of 8 fused 128 KB ones —
    # engaging all 16 SDMA engines instead of 8. Load and store use the same
    # view, so it is a pure (correct) permutation of positions.
    xv = x.rearrange("n t d -> n (t d)").rearrange(
        "n (g c f) -> c n g f", c=NT, g=G, f=F
    )
    yv = y.rearrange("n t d -> n (t d)").rearrange(
        "n (g c f) -> c n g f", c=NT, g=G, f=F
    )

    with TileContext(nc) as tc:
        with (
            tc.tile_pool(name="wp", bufs=1) as wp,
            tc.tile_pool(name="xp", bufs=4) as xp,
            tc.tile_pool(name="hp", bufs=3) as hp,
            tc.tile_pool(name="lp", bufs=3) as lp,
            tc.tile_pool(name="yp", bufs=4) as yp,
            tc.tile_pool(name="pp", bufs=8, space="PSUM") as pp,
        ):
            if USE_BF16:
                wht = wp.tile([128, 128], BF16)
                nc.sync.dma_start(wht[:], wh[:])
                wlt = wp.tile([128, 128], BF16)
                nc.sync.dma_start(wlt[:], wl[:])
            else:
                wt = wp.tile([128, 128], F32)
                nc.sync.dma_start(wt[:], w[:])
            for c in range(NT):
                # Alternate the two HWDGE rings (SP / ACT sequencers) between
                # input and output streams for queue-level DMA parallelism.
                eng_in = nc.sync if c % 2 == 0 else nc.scalar
                eng_out = nc.scalar if c % 2 == 0 else nc.sync
                xt = xp.tile([128, F], F32)
                # dst is plain [128, F]; src [n, g, f] enumerates elements in
                # partition order (p = n*G + g) — the DMA matches element order.
                eng_in.dma_start(xt[:], xv[c])
                yt = yp.tile([128, F], F32)
                if USE_BF16:
                    # Split x = xh + xl (bf16 each, ~2^-17 exact together):
                    # cast on ACT, residual on DVE.
                    xh = hp.tile([128, F], BF16)
                    nc.scalar.copy(xh[:], xt[:])
                    xl = lp.tile([128, F], BF16)
                    nc.vector.tensor_sub(xl[:], xt[:], xh[:])
                    for k in range(F // MM_N):
                        sl = slice(k * MM_N, (k + 1) * MM_N)
                        ps = pp.tile([128, MM_N], F32)
                        nc.tensor.matmul(
                            ps[:], wht[:], xh[:, sl], start=True, stop=False
                        )
                        nc.tensor.matmul(
                            ps[:], wht[:], xl[:, sl], start=False, stop=False
                        )
                        nc.tensor.matmul(
                            ps[:], wlt[:], xh[:, sl], start=False, stop=True
                        )
                        # Split PSUM->SBUF copies 3:1 between DVE and ACT.
                        if k % 4 == 3:
                            nc.scalar.copy(yt[:, sl], ps[:])
                        else:
                            nc.vector.tensor_copy(yt[:, sl], ps[:])
                else:
                    for k in range(F // MM_N):
                        sl = slice(k * MM_N, (k + 1) * MM_N)
                        ps = pp.tile([128, MM_N], F32)
                        nc.tensor.matmul(
                            ps[:], wt[:], xt[:, sl], start=True, stop=True
                        )
                        if k % 4 == 3:
                            nc.scalar.copy(yt[:, sl], ps[:])
                        else:
                            nc.vector.tensor_copy(yt[:, sl], ps[:])
                eng_out.dma_start(yv[c], yt[:])
    nc.compile()
    return nc


def kernel(streams, logits):
    streams = np.asarray(streams, dtype=np.float32)
    logits = np.asarray(logits, dtype=np.float32)

    temp = np.float32(max(TEMPERATURE, 1e-6))
    H = _sinkhorn_np(logits / temp)
    Wm = _expand_w(H)

    if "nc" not in _cache:
        _cache["nc"] = _build_nc()
    nc = _cache["nc"]

    if USE_BF16:
        Wh = Wm.astype(np.float16)
        Wl = (Wm - Wh.astype(np.float32)).astype(np.float16)

    in_maps = []
    for c in range(8):
        b, th = divmod(c, 2)
        xc = np.ascontiguousarray(streams[b, :, th * TH : (th + 1) * TH, :])
        if USE_BF16:
            in_maps.append({"x": xc, "wh": Wh, "wl": Wl})
        else:
            in_maps.append({"x": xc, "w": Wm})

    trace = os.environ.get("MIX_TRACE", "") == "1"
    if trace:
        _install_ntff_hook()
    res = bass_utils.run_bass_kernel_spmd(
        nc,
        in_maps,
        list(range(8)),
        trace=trace,
        tmpdir=os.environ.get("MIX_TMPDIR") or None,
    )
    _cache["last_results"] = res

    out = np.empty((B, N, T, D), dtype=np.float32)
    for c in range(8):
        b, th = divmod(c, 2)
        out[b, :, th * TH : (th + 1) * TH, :] = res.results[c]["y"]
    return out



# revision 2
# speedup vs baseline: 1.9950x; 1.9950x over previous
"""MHC residual mixer: out[b,i,t,d] = sum_j H[i,j] * streams[b,j,t,d],
H = sinkhorn(logits). Streams mixed on-device; Sinkhorn (8x8, 20 iters) on host.

Sharding: 8 cores, core c handles batch b=c//2, T-half c%2 -> per-core
x[8, 1024, 1024] (16 MiB fp16). The stream-mix becomes a [128,128] stationary
matmul by packing (stream j, group g) on partitions and using a block-diagonal
weight W[j*16+g, i*16+g] = H[i,j].

Wire format is fp16 end-to-end (x, W, y): the problem is HBM-bound (per-core
roofline 358 GB/s), so halving the bytes halves the time; fp16 also runs the
PE at full rate (fp32 matmul is quarter-rate). Host casts cost ~1.5e-3 max
abs err against a 5.4 output scale -- far inside the 2e-2 gate.
"""

import os
import sys
import types
import numpy as np

import concourse.bass as bass
import concourse.mybir as mybir
from concourse import bacc
from concourse import bass_utils
from concourse.tile import TileContext


def _install_ntff_hook():
    # The image's `antenv` package lacks `axon_hooks`, so bass_utils'
    # trace path can't find the NTFF profile hook. Recreate it from the
    # boot shim's ctypes factory. Only needed when profiling (MIX_TRACE=1).
    if "antenv.axon_hooks" in sys.modules:
        return
    try:
        import antenv
        from trn_agent_boot.trn_boot import _ntff_profile_via_ctypes

        hook = _ntff_profile_via_ctypes("/opt/axon/libaxon_pjrt.so")
        mod = types.ModuleType("antenv.axon_hooks")
        mod.get_axon_ntff_profile_hook = lambda: hook
        mod.set_axon_ntff_profile_hook = lambda h: None
        sys.modules["antenv.axon_hooks"] = mod
        antenv.axon_hooks = mod
    except Exception as e:  # profiling is best-effort; execution still works
        print(f"ntff hook install failed: {e}", file=sys.stderr)

B, N, T, D = 4, 8, 2048, 1024
TH = T // 2                      # per-core T slice
POS = TH * D                     # positions per core per stream = 1,048,576
G = 16                           # groups on partitions (N*G = 128)
F = 4096                         # free columns per SBUF tile
MM_N = 512                       # PSUM-bank-limited moving free dim
NT = POS // (G * F)              # tiles per core
SINKHORN_ITERS = 20
TEMPERATURE = 1.0
EPS = np.float32(1e-8)
F32 = mybir.dt.float32
F16 = mybir.dt.float16

_cache = {}


def _sinkhorn_np(logits):
    x = logits.astype(np.float32)
    x = x - x.max(axis=-1, keepdims=True)
    p = np.exp(x) + EPS
    for _ in range(SINKHORN_ITERS):
        p = p / (p.sum(axis=-1, keepdims=True) + EPS)
        p = p / (p.sum(axis=-2, keepdims=True) + EPS)
    return p.astype(np.float32)


def _expand_w(H):
    # W[j*G+g, i*G+g] = H[i, j]  so that  out = W.T @ x  mixes streams per group
    Wm = np.zeros((128, 128), dtype=np.float32)
    g = np.arange(G)
    for j in range(N):
        for i in range(N):
            Wm[j * G + g, i * G + g] = H[i, j]
    return Wm


def _build_nc():
    nc = bacc.Bacc(
        "TRN2", target_bir_lowering=False, debug=False, enable_asserts=False
    )
    x = nc.dram_tensor("x", [N, TH, D], F16, kind="ExternalInput").ap()
    w = nc.dram_tensor("w", [128, 128], F16, kind="ExternalInput").ap()
    y = nc.dram_tensor("y", [N, TH, D], F16, kind="ExternalOutput").ap()

    # g-major position layout: position = g*(NT*F) + c*F + f. The 16 g-chunks
    # per stream are non-adjacent in DRAM, so each per-tile DMA lowers to
    # 128 descriptors of F*2 bytes (8 KB) spread across all 16 SDMA engines.
    # Load and store use the same view, so it is a pure permutation.
    xv = x.rearrange("n t d -> n (t d)").rearrange(
        "n (g c f) -> c n g f", c=NT, g=G, f=F
    )
    yv = y.rearrange("n t d -> n (t d)").rearrange(
        "n (g c f) -> c n g f", c=NT, g=G, f=F
    )

    with TileContext(nc) as tc:
        with (
            tc.tile_pool(name="wp", bufs=1) as wp,
            tc.tile_pool(name="xp", bufs=4) as xp,
            tc.tile_pool(name="yp", bufs=4) as yp,
            tc.tile_pool(name="pp", bufs=8, space="PSUM") as pp,
        ):
            wt = wp.tile([128, 128], F16)
            nc.sync.dma_start(wt[:], w[:])
            for c in range(NT):
                # Alternate the two HWDGE rings (SP / ACT sequencers) between
                # input and output streams for queue-level DMA parallelism.
                eng_in = nc.sync if c % 2 == 0 else nc.scalar
                eng_out = nc.scalar if c % 2 == 0 else nc.sync
                xt = xp.tile([128, F], F16)
                # dst is plain [128, F]; src [n, g, f] enumerates elements in
                # partition order (p = n*G + g) — the DMA matches element order.
                eng_in.dma_start(xt[:], xv[c])
                yt = yp.tile([128, F], F16)
                for k in range(F // MM_N):
                    sl = slice(k * MM_N, (k + 1) * MM_N)
                    ps = pp.tile([128, MM_N], F32)
                    nc.tensor.matmul(
                        ps[:], wt[:], xt[:, sl], start=True, stop=True
                    )
                    # Split PSUM->SBUF downcast copies 3:1 between DVE and ACT.
                    if k % 4 == 3:
                        nc.scalar.copy(yt[:, sl], ps[:])
                    else:
                        nc.vector.tensor_copy(yt[:, sl], ps[:])
                eng_out.dma_start(yv[c], yt[:])
    nc.compile()
    return nc


def kernel(streams, logits):
    streams = np.asarray(streams, dtype=np.float32)
    logits = np.asarray(logits, dtype=np.float32)

    temp = np.float32(max(TEMPERATURE, 1e-6))
    H = _sinkhorn_np(logits / temp)
    W16 = _expand_w(H).astype(np.float16)

    if "nc" not in _cache:
        _cache["nc"] = _build_nc()
    nc = _cache["nc"]

    s16 = streams.astype(np.float16)
    in_maps = []
    for c in range(8):
        b, th = divmod(c, 2)
        xc = np.ascontiguousarray(s16[b, :, th * TH : (th + 1) * TH, :])
        in_maps.append({"x": xc, "w": W16})

    trace = os.environ.get("MIX_TRACE", "") == "1"
    if trace:
        _install_ntff_hook()
    res = bass_utils.run_bass_kernel_spmd(
        nc,
        in_maps,
        list(range(8)),
        trace=trace,
        tmpdir=os.environ.get("MIX_TMPDIR") or None,
    )
    _cache["last_results"] = res

    out = np.empty((B, N, T, D), dtype=np.float32)
    for c in range(8):
        b, th = divmod(c, 2)
        out[b, :, th * TH : (th + 1) * TH, :] = res.results[c]["y"]
    return out


# revision 3
# speedup vs baseline: 2.1672x; 1.0863x over previous
"""MHC residual mixer: out[b,i,t,d] = sum_j H[i,j] * streams[b,j,t,d],
H = sinkhorn(logits). Streams mixed on-device; Sinkhorn (8x8, 20 iters) on host.

Sharding: 8 cores, core c handles batch b=c//2, T-half c%2 -> per-core
x[8, 1024, 1024] (16 MiB fp16). The stream-mix becomes a [128,128] stationary
matmul by packing (stream j, group g) on partitions and using a block-diagonal
weight W[j*16+g, i*16+g] = H[i,j].

Wire format is fp16 end-to-end (x, W, y): the problem is HBM-bound (per-core
roofline 358 GB/s), so halving the bytes halves the time; fp16 also runs the
PE at full rate (fp32 matmul is quarter-rate). Host casts cost ~1.5e-3 max
abs err against a 5.4 output scale -- far inside the 2e-2 gate.
"""

import os
import sys
import types
import numpy as np

import concourse.bass as bass
import concourse.mybir as mybir
from concourse import bacc
from concourse import bass_utils
from concourse.tile import TileContext


def _install_ntff_hook():
    # The image's `antenv` package lacks `axon_hooks`, so bass_utils'
    # trace path can't find the NTFF profile hook. Recreate it from the
    # boot shim's ctypes factory. Only needed when profiling (MIX_TRACE=1).
    if "antenv.axon_hooks" in sys.modules:
        return
    try:
        import antenv
        from trn_agent_boot.trn_boot import _ntff_profile_via_ctypes

        hook = _ntff_profile_via_ctypes("/opt/axon/libaxon_pjrt.so")
        mod = types.ModuleType("antenv.axon_hooks")
        mod.get_axon_ntff_profile_hook = lambda: hook
        mod.set_axon_ntff_profile_hook = lambda h: None
        sys.modules["antenv.axon_hooks"] = mod
        antenv.axon_hooks = mod
    except Exception as e:  # profiling is best-effort; execution still works
        print(f"ntff hook install failed: {e}", file=sys.stderr)

B, N, T, D = 4, 8, 2048, 1024
TH = T // 2                      # per-core T slice
POS = TH * D                     # positions per core per stream = 1,048,576
G = 16                           # groups on partitions (N*G = 128)
F = 4096                         # free columns per SBUF tile
MM_N = 512                       # PSUM-bank-limited moving free dim
NT = POS // (G * F)              # tiles per core
SINKHORN_ITERS = 20
TEMPERATURE = 1.0
EPS = np.float32(1e-8)
F32 = mybir.dt.float32
F16 = mybir.dt.float16

_cache = {}


def _sinkhorn_np(logits):
    x = logits.astype(np.float32)
    x = x - x.max(axis=-1, keepdims=True)
    p = np.exp(x) + EPS
    for _ in range(SINKHORN_ITERS):
        p = p / (p.sum(axis=-1, keepdims=True) + EPS)
        p = p / (p.sum(axis=-2, keepdims=True) + EPS)
    return p.astype(np.float32)


def _expand_w(H):
    # W[j*G+g, i*G+g] = H[i, j]  so that  out = W.T @ x  mixes streams per group
    Wm = np.zeros((128, 128), dtype=np.float32)
    g = np.arange(G)
    for j in range(N):
        for i in range(N):
            Wm[j * G + g, i * G + g] = H[i, j]
    return Wm


def _build_nc():
    nc = bacc.Bacc(
        "TRN2", target_bir_lowering=False, debug=False, enable_asserts=False
    )
    x = nc.dram_tensor("x", [N, TH, D], F16, kind="ExternalInput").ap()
    w = nc.dram_tensor("w", [128, 128], F16, kind="ExternalInput").ap()
    y = nc.dram_tensor("y", [N, TH, D], F16, kind="ExternalOutput").ap()

    # g-major position layout: position = g*(NT*F) + c*F + f. The 16 g-chunks
    # per stream are non-adjacent in DRAM, so each per-tile DMA lowers to
    # 128 descriptors of F*2 bytes (8 KB) spread across all 16 SDMA engines.
    # Load and store use the same view, so it is a pure permutation.
    xv = x.rearrange("n t d -> n (t d)").rearrange(
        "n (g c f) -> c n g f", c=NT, g=G, f=F
    )
    yv = y.rearrange("n t d -> n (t d)").rearrange(
        "n (g c f) -> c n g f", c=NT, g=G, f=F
    )

    with TileContext(nc) as tc:
        with (
            tc.tile_pool(name="wp", bufs=1) as wp,
            tc.tile_pool(name="xp", bufs=6) as xp,
            tc.tile_pool(name="yp", bufs=4) as yp,
            tc.tile_pool(name="pp", bufs=8, space="PSUM") as pp,
        ):
            wt = wp.tile([128, 128], F16)
            nc.scalar.dma_start(wt[:], w[:])
            for c in range(NT):
                # Loads ride the SP ring, stores the ACT ring: HWDGE rings are
                # FIFO per sequencer, so a store stalled on compute must never
                # queue ahead of the next load.
                xt = xp.tile([128, F], F16)
                # dst is plain [128, F]; src [n, g, f] enumerates elements in
                # partition order (p = n*G + g) — the DMA matches element order.
                nc.sync.dma_start(xt[:], xv[c])
                yt = yp.tile([128, F], F16)
                for k in range(F // MM_N):
                    sl = slice(k * MM_N, (k + 1) * MM_N)
                    ps = pp.tile([128, MM_N], F32)
                    nc.tensor.matmul(
                        ps[:], wt[:], xt[:, sl], start=True, stop=True
                    )
                    # Split PSUM->SBUF downcast copies 1:1 between DVE and ACT.
                    if k % 2 == 1:
                        nc.scalar.copy(yt[:, sl], ps[:])
                    else:
                        nc.vector.tensor_copy(yt[:, sl], ps[:])
                nc.scalar.dma_start(yv[c], yt[:])
    nc.compile()
    return nc


def kernel(streams, logits):
    streams = np.asarray(streams, dtype=np.float32)
    logits = np.asarray(logits, dtype=np.float32)

    temp = np.float32(max(TEMPERATURE, 1e-6))
    H = _sinkhorn_np(logits / temp)
    W16 = _expand_w(H).astype(np.float16)

    if "nc" not in _cache:
        _cache["nc"] = _build_nc()
    nc = _cache["nc"]

    s16 = streams.astype(np.float16)
    in_maps = []
    for c in range(8):
        b, th = divmod(c, 2)
        xc = np.ascontiguousarray(s16[b, :, th * TH : (th + 1) * TH, :])
        in_maps.append({"x": xc, "w": W16})

    trace = os.environ.get("MIX_TRACE", "") == "1"
    if trace:
        _install_ntff_hook()
    res = bass_utils.run_bass_kernel_spmd(
        nc,
        in_maps,
        list(range(8)),
        trace=trace,
        tmpdir=os.environ.get("MIX_TMPDIR") or None,
    )
    _cache["last_results"] = res

    out = np.empty((B, N, T, D), dtype=np.float32)
    for c in range(8):
        b, th = divmod(c, 2)
        out[b, :, th * TH : (th + 1) * TH, :] = res.results[c]["y"]
    return out
